# revision 6
# baseline (speedup 1.0000x reference)
"""Trainium2 Bass kernel for CosineGraphAttentionLayer (v2: minimal-bytes).

reference:
    cos = beta * (xi @ xj.T) / (|xi| |xj| + eps)
    P   = softmax(cos + (1-adj) * -1e9, axis=1)
    out = P @ xj

Measured bottleneck of v1 was per-iteration input shipping (~350 MB across
8 cores), not device compute. v2 minimizes shipped bytes:
  - adj (int32, 268 MB total) -> adj.T bit-packed on host (8 MB total),
    unpacked on-device with one DVE bitwise-and against a per-column
    bit-select row. The AND yields {0, 2^(i%8)} instead of {0,1}; the
    2^(i%8) factor is constant per output row i, so it cancels exactly
    between the numerator and the softmax denominator (both accumulated
    in MM2) - powers of two, no rounding impact.
  - xi / xj shipped as bf16 (matmuls run bf16 anyway: 1 PE cycle/row vs 4
    for fp32).
  - output returned as bf16, upcast to f32 on host.

Per-core kernel (scores kept TRANSPOSED [j, i], like v1):
  - fold beta/|xi| into xi rows and 1/|xj| into xj rows (eps dropped:
    relative effect ~eps/D ~ 4e-10)
  - MM1: ST[j,i] = xj_s @ xi_s.T via PE (both operands pre-transposed
    on-chip with PE transposes, bf16)
  - E = exp(ST) on ACT from PSUM (|ST| <= beta <= 1: no max subtraction)
  - mask: m = adjT_bytes(rep8) & bitsel (DVE, uint8); pu = E * m (bf16)
  - MM2: out[i, 0:256] += pu_tile.T @ [xj | 1] over all j in PSUM;
    column 256 accumulates the softmax denominator
  - normalize rows by 1/den on DVE, DMA out as bf16
"""
import sys

sys.path.insert(0, "/opt/trn_rl_repo")

import numpy as np
import ml_dtypes

import concourse.bass as bass
import concourse.bacc as bacc
import concourse.tile as tile
from concourse import mybir, masks
from concourse.bass_utils import run_bass_kernel_spmd

F32 = mybir.dt.float32
BF16 = mybir.dt.bfloat16
U8 = mybir.dt.uint8

N_CORES = 8


def build_nc(NI=1024, M=8192, D=256):
    """Per-core bass program. NI = rows per core, M = columns (j), D = feature dim."""
    assert NI % 256 == 0 and M % 1024 == 0 and D == 256
    NIB = NI // 128          # i-blocks per core
    NHALF = 2                # i halves (PSUM capacity)
    IBH = NIB // NHALF       # i-blocks per half
    IW = IBH * 128           # i width per half
    IWB = IW // 8            # bytes of packed adjT per half-row
    NJB = M // 128           # j blocks
    DH = D // 128            # d halves
    NB = NI // 8             # packed bytes per adjT row

    MS = M // N_CORES        # xj shard rows per core

    nc = bacc.Bacc("TRN2", target_bir_lowering=False, debug=False)
    xi = nc.declare_dram_parameter("xi", [NI, D], BF16, isOutput=False)
    xjs = nc.declare_dram_parameter("xjs", [MS, D], BF16, isOutput=False)
    adjp = nc.declare_dram_parameter("adjp", [M, NB], U8, isOutput=False)
    bitsel = nc.declare_dram_parameter("bitsel", [NI], U8, isOutput=False)
    beta = nc.declare_dram_parameter("beta", [1], F32, isOutput=False)
    out = nc.declare_dram_parameter("out", [NI, D], BF16, isOutput=True)
    # xj arrives sharded (MS rows per core); AllGather reassembles the full
    # [M, D] on-device so only 1/N_CORES of xj ships per iteration.
    xj_stage = nc.dram_tensor("xj_stage", [MS, D], BF16)
    xj = nc.dram_tensor("xj_full", [M, D], BF16, addr_space="Shared")

    with tile.TileContext(nc) as tc:
        with (
            tc.tile_pool(name="big", bufs=1) as big,
            tc.tile_pool(name="prep", bufs=3) as prep,
            tc.tile_pool(name="work", bufs=3) as work,
            tc.tile_pool(name="outp", bufs=4) as outp,
            tc.tile_pool(name="ps_s", bufs=2, space="PSUM") as ps_s,
            tc.tile_pool(name="ps_o", bufs=IBH, space="PSUM") as ps_o,
            tc.tile_pool(name="ps_t", bufs=2, space="PSUM") as ps_t,
        ):
            # ---------------- static tiles ----------------
            xj_aug = [big.tile([128, 8, D + 1], BF16, name=f"xj_aug{g}", tag=f"xj_aug{g}")
                      for g in range(NJB // 8)]
            xj_sT = [big.tile([128, M], BF16, name=f"xj_sT{dh}", tag=f"xj_sT{dh}") for dh in range(DH)]
            xi_sT = [big.tile([128, NI], BF16, name=f"xi_sT{dh}", tag=f"xi_sT{dh}") for dh in range(DH)]
            ident = big.tile([128, 128], BF16)
            beta_sb = big.tile([128, 1], F32)
            adjp_sb = big.tile([128, NJB, NB], U8)
            bits_sb = big.tile([128, NI], U8)
            ssq_j = big.tile([128, NJB], F32)
            ssq_i = big.tile([128, NIB], F32)
            rj = big.tile([128, NJB], F32)
            ri = big.tile([128, NIB], F32)

            masks.make_identity(nc, ident[:, :])
            nc.scalar.dma_start(
                out=beta_sb[:, :],
                in_=bass.AP(tensor=beta, offset=0, ap=[[0, 128], [1, 1]]),
            )
            nc.scalar.dma_start(
                out=bits_sb[:, :],
                in_=bass.AP(tensor=bitsel, offset=0, ap=[[0, 128], [1, NI]]),
            )

            # ---------------- gather xj shards ----------------
            xjs_r = xjs[:, :].rearrange("(jb p) d -> p jb d", p=128)
            stage_r = xj_stage[:, :].rearrange("(jb p) d -> p jb d", p=128)
            tstage = big.tile([128, MS // 128, D], BF16)
            nc.scalar.dma_start(out=tstage[:, :, :], in_=xjs_r[:, :, :])
            nc.scalar.dma_start(out=stage_r[:, :, :], in_=tstage[:, :, :])
            nc.gpsimd.collective_compute(
                kind="AllGather",
                op=mybir.AluOpType.bypass,
                replica_groups=[list(range(N_CORES))],
                ins=[xj_stage[:, :]],
                outs=[xj[:, :]],
            )

            # ---------------- prep: load, norms, scale, transpose ----------------
            xj_r = xj[:, :].rearrange("(jb p) d -> p jb d", p=128)
            xi_r = xi[:, :].rearrange("(ib p) d -> p ib d", p=128)
            adjp_r = adjp[:, :].rearrange("(jb p) c -> p jb c", p=128)
            for g in range(NJB // 8):
                nc.scalar.dma_start(
                    out=xj_aug[g][:, :, 0:D], in_=xj_r[:, 8 * g:8 * (g + 1), :]
                )
                nc.vector.memset(xj_aug[g][:, :, D:D + 1], 1.0)
            nc.scalar.dma_start(out=adjp_sb[:, :, :], in_=adjp_r[:, :, :])

            xi_all = big.tile([128, NIB, D], BF16)
            nc.scalar.dma_start(out=xi_all[:, :, :], in_=xi_r[:, :, :])

            # row sums of squares via ACT Square + accum_out
            for jb in range(NJB):
                sq = prep.tile([128, D], F32, tag="sq")
                nc.scalar.activation(
                    out=sq[:, :], in_=xj_aug[jb // 8][:, jb % 8, 0:D],
                    func=mybir.ActivationFunctionType.Square,
                    accum_out=ssq_j[:, jb:jb + 1],
                )
            for ib in range(NIB):
                sq = prep.tile([128, D], F32, tag="sq")
                nc.scalar.activation(
                    out=sq[:, :], in_=xi_all[:, ib, :],
                    func=mybir.ActivationFunctionType.Square,
                    accum_out=ssq_i[:, ib:ib + 1],
                )
            # rj = 1/sqrt(ssq_j); ri = beta/sqrt(ssq_i)
            nc.scalar.activation(out=ssq_j[:, :], in_=ssq_j[:, :],
                                 func=mybir.ActivationFunctionType.Sqrt)
            nc.vector.reciprocal(out=rj[:, :], in_=ssq_j[:, :])
            nc.scalar.activation(out=ssq_i[:, :], in_=ssq_i[:, :],
                                 func=mybir.ActivationFunctionType.Sqrt)
            nc.vector.reciprocal(out=ri[:, :], in_=ssq_i[:, :])
            nc.vector.tensor_scalar(out=ri[:, :], in0=ri[:, :],
                                    scalar1=beta_sb[:, 0:1], scalar2=None,
                                    op0=mybir.AluOpType.mult)

            # scale rows then PE-transpose into xj_sT / xi_sT
            for jb in range(NJB):
                t = prep.tile([128, D], BF16, tag="xjs")
                nc.vector.tensor_scalar(out=t[:, :], in0=xj_aug[jb // 8][:, jb % 8, 0:D],
                                        scalar1=rj[:, jb:jb + 1], scalar2=None,
                                        op0=mybir.AluOpType.mult)
                for dh in range(DH):
                    tp = ps_t.tile([128, 128], BF16, tag="tp")
                    nc.tensor.matmul(tp[:, :], t[:, 128 * dh:128 * (dh + 1)],
                                     ident[:, :], is_transpose=True)
                    nc.vector.tensor_copy(
                        xj_sT[dh][:, 128 * jb:128 * (jb + 1)], tp[:, :])
            for ib in range(NIB):
                t = prep.tile([128, D], BF16, tag="xis")
                nc.vector.tensor_scalar(out=t[:, :], in0=xi_all[:, ib, :],
                                        scalar1=ri[:, ib:ib + 1], scalar2=None,
                                        op0=mybir.AluOpType.mult)
                for dh in range(DH):
                    tp = ps_t.tile([128, 128], BF16, tag="tp")
                    nc.tensor.matmul(tp[:, :], t[:, 128 * dh:128 * (dh + 1)],
                                     ident[:, :], is_transpose=True)
                    nc.vector.tensor_copy(
                        xi_sT[dh][:, 128 * ib:128 * (ib + 1)], tp[:, :])

            # ---------------- main loop ----------------
            for h in range(NHALF):
                ps_out = [ps_o.tile([128, D + 1], F32, name=f"ps_out_{h}_{bb}", tag="ps_out")
                          for bb in range(IBH)]
                for jb in range(NJB):
                    # MM1: ST[j=128, i=IW]
                    st = ps_s.tile([128, IW], F32, tag="st")
                    for dh in range(DH):
                        nc.tensor.matmul(
                            st[:, :],
                            xj_sT[dh][:, 128 * jb:128 * (jb + 1)],
                            xi_sT[dh][:, IW * h:IW * (h + 1)],
                            start=(dh == 0), stop=(dh == DH - 1),
                        )
                    e = work.tile([128, IW], F32, tag="e")
                    nc.scalar.activation(
                        out=e[:, :], in_=st[:, :],
                        func=mybir.ActivationFunctionType.Exp)
                    # mask bits: m[j, 8c+k] = adjT_byte[j, c] & (1 << k)
                    m = work.tile([128, IW], U8, tag="m")
                    byte_ap = adjp_sb[:, jb, h * IWB:(h + 1) * IWB]
                    rep8 = byte_ap.unsqueeze(2).broadcast_to([128, IWB, 8])
                    nc.vector.tensor_tensor(
                        out=m[:, :].rearrange("p (c k) -> p c k", k=8),
                        in0=rep8,
                        in1=bits_sb[:, IW * h:IW * (h + 1)].rearrange(
                            "p (c k) -> p c k", k=8),
                        op=mybir.AluOpType.bitwise_and,
                    )
                    # pu = e * m  (m in {0, 2^k}; the 2^k column factor cancels
                    # against the denominator in the final normalization)
                    pu = work.tile([128, IW], BF16, tag="pu")
                    nc.vector.tensor_tensor(
                        out=pu[:, :], in0=e[:, :], in1=m[:, :],
                        op=mybir.AluOpType.mult,
                    )
                    # MM2: out[i, :] += pu_tile.T @ xj_aug
                    for b in range(IBH):
                        nc.tensor.matmul(
                            ps_out[b][:, :],
                            pu[:, 128 * b:128 * (b + 1)],
                            xj_aug[jb // 8][:, jb % 8, :],
                            start=(jb == 0), stop=(jb == NJB - 1),
                        )
                # normalize + store
                for b in range(IBH):
                    ib = h * IBH + b
                    rden = outp.tile([128, 1], F32, tag="rden")
                    nc.vector.reciprocal(out=rden[:, :], in_=ps_out[b][:, D:D + 1])
                    of = outp.tile([128, D], BF16, tag="of")
                    nc.vector.tensor_scalar(
                        out=of[:, :], in0=ps_out[b][:, 0:D],
                        scalar1=rden[:, 0:1], scalar2=None,
                        op0=mybir.AluOpType.mult)
                    nc.scalar.dma_start(
                        out=out[128 * ib:128 * (ib + 1), :], in_=of[:, :])

    nc.finalize()
    return nc


_NC_CACHE = {}


def _get_nc(NI, M, D):
    key = (NI, M, D)
    if key not in _NC_CACHE:
        _NC_CACHE[key] = build_nc(NI, M, D)
    return _NC_CACHE[key]


def _prep_inputs(xi, xj, adj, beta):
    """Host-side layout/dtype prep (not device math): bf16 casts + bit-pack."""
    xi16 = np.asarray(xi, dtype=np.float32).astype(ml_dtypes.bfloat16)
    xj16 = np.asarray(xj, dtype=np.float32).astype(ml_dtypes.bfloat16)
    adjT = np.ascontiguousarray(np.asarray(adj, dtype=np.int32).T.astype(np.uint8))
    adjp = np.packbits(adjT, axis=1, bitorder="little")  # [M, N/8]
    beta = np.ascontiguousarray(np.asarray(beta, dtype=np.float32))
    return xi16, xj16, adjp, beta


def _make_in_maps(xi, xj, adj, beta):
    N, D = xi.shape
    NI = N // N_CORES
    xi16, xj16, adjp, beta = _prep_inputs(xi, xj, adj, beta)
    bitsel = np.tile(np.left_shift(np.ones(8, np.uint8), np.arange(8, dtype=np.uint8)),
                     NI // 8)
    M = xj16.shape[0]
    MS = M // N_CORES
    return [
        {
            "xi": xi16[k * NI:(k + 1) * NI],
            "xjs": xj16[k * MS:(k + 1) * MS],
            "adjp": np.ascontiguousarray(adjp[:, k * (NI // 8):(k + 1) * (NI // 8)]),
            "bitsel": bitsel,
            "beta": beta,
        }
        for k in range(N_CORES)
    ]


def kernel(xi, xj, adj, beta):
    N, D = xi.shape
    M = xj.shape[0]
    NI = N // N_CORES
    nc = _get_nc(NI, M, D)
    in_maps = _make_in_maps(xi, xj, adj, beta)
    res = run_bass_kernel_spmd(nc, in_maps, list(range(N_CORES)))
    out16 = np.concatenate([res.results[k]["out"] for k in range(N_CORES)], axis=0)
    return out16.astype(np.float32)


# revision 7
# speedup vs baseline: 1.4577x; 1.4577x over previous
"""Trainium2 Bass kernel for CosineGraphAttentionLayer (v2: minimal-bytes).

reference:
    cos = beta * (xi @ xj.T) / (|xi| |xj| + eps)
    P   = softmax(cos + (1-adj) * -1e9, axis=1)
    out = P @ xj

Measured bottleneck of v1 was per-iteration input shipping (~350 MB across
8 cores), not device compute. v2 minimizes shipped bytes:
  - adj (int32, 268 MB total) -> adj.T bit-packed on host (8 MB total),
    unpacked on-device with one DVE bitwise-and against a per-column
    bit-select row. The AND yields {0, 2^(i%8)} instead of {0,1}; the
    2^(i%8) factor is constant per output row i, so it cancels exactly
    between the numerator and the softmax denominator (both accumulated
    in MM2) - powers of two, no rounding impact.
  - xi / xj shipped as bf16 (matmuls run bf16 anyway: 1 PE cycle/row vs 4
    for fp32).
  - output returned as bf16, upcast to f32 on host.

Per-core kernel (scores kept TRANSPOSED [j, i], like v1):
  - fold beta/|xi| into xi rows and 1/|xj| into xj rows (eps dropped:
    relative effect ~eps/D ~ 4e-10)
  - MM1: ST[j,i] = xj_s @ xi_s.T via PE (both operands pre-transposed
    on-chip with PE transposes, bf16)
  - E = exp(ST) on ACT from PSUM (|ST| <= beta <= 1: no max subtraction)
  - mask: m = adjT_bytes(rep8) & bitsel (DVE, uint8); pu = E * m (bf16)
  - MM2: out[i, 0:256] += pu_tile.T @ [xj | 1] over all j in PSUM;
    column 256 accumulates the softmax denominator
  - normalize rows by 1/den on DVE, DMA out as bf16
"""
import sys

sys.path.insert(0, "/opt/trn_rl_repo")

import numpy as np
import ml_dtypes

import concourse.bass as bass
import concourse.bacc as bacc
import concourse.tile as tile
from concourse import mybir, masks
from concourse.bass_utils import run_bass_kernel_spmd

F32 = mybir.dt.float32
BF16 = mybir.dt.bfloat16
U8 = mybir.dt.uint8

N_CORES = 8


def build_nc(NI=1024, M=8192, D=256):
    """Per-core bass program. NI = rows per core, M = columns (j), D = feature dim."""
    assert NI % 256 == 0 and M % 1024 == 0 and D == 256
    NIB = NI // 128          # i-blocks per core
    NHALF = 2                # i halves (PSUM capacity)
    IBH = NIB // NHALF       # i-blocks per half
    IW = IBH * 128           # i width per half
    IWB = IW // 8            # bytes of packed adjT per half-row
    NJB = M // 128           # j blocks
    DH = D // 128            # d halves
    NB = NI // 8             # packed bytes per adjT row

    nc = bacc.Bacc("TRN2", target_bir_lowering=False, debug=False)
    xi = nc.declare_dram_parameter("xi", [NI, D], BF16, isOutput=False)
    xj = nc.declare_dram_parameter("xj", [M, D], BF16, isOutput=False)
    adjp = nc.declare_dram_parameter("adjp", [M, NB], U8, isOutput=False)
    bitsel = nc.declare_dram_parameter("bitsel", [NI], U8, isOutput=False)
    beta = nc.declare_dram_parameter("beta", [1], F32, isOutput=False)
    out = nc.declare_dram_parameter("out", [NI, D], BF16, isOutput=True)

    with tile.TileContext(nc) as tc:
        with (
            tc.tile_pool(name="big", bufs=1) as big,
            tc.tile_pool(name="prep", bufs=3) as prep,
            tc.tile_pool(name="work", bufs=3) as work,
            tc.tile_pool(name="outp", bufs=4) as outp,
            tc.tile_pool(name="ps_s", bufs=2, space="PSUM") as ps_s,
            tc.tile_pool(name="ps_o", bufs=IBH, space="PSUM") as ps_o,
            tc.tile_pool(name="ps_t", bufs=2, space="PSUM") as ps_t,
        ):
            # ---------------- static tiles ----------------
            xj_aug = [big.tile([128, 8, D + 1], BF16, name=f"xj_aug{g}", tag=f"xj_aug{g}")
                      for g in range(NJB // 8)]
            xj_sT = [big.tile([128, M], BF16, name=f"xj_sT{dh}", tag=f"xj_sT{dh}") for dh in range(DH)]
            xi_sT = [big.tile([128, NI], BF16, name=f"xi_sT{dh}", tag=f"xi_sT{dh}") for dh in range(DH)]
            ident = big.tile([128, 128], BF16)
            beta_sb = big.tile([128, 1], F32)
            adjp_sb = big.tile([128, NJB, NB], U8)
            bits_sb = big.tile([128, NI], U8)
            ssq_j = big.tile([128, NJB], F32)
            ssq_i = big.tile([128, NIB], F32)
            rj = big.tile([128, NJB], F32)
            ri = big.tile([128, NIB], F32)

            masks.make_identity(nc, ident[:, :])
            nc.scalar.dma_start(
                out=beta_sb[:, :],
                in_=bass.AP(tensor=beta, offset=0, ap=[[0, 128], [1, 1]]),
            )
            nc.scalar.dma_start(
                out=bits_sb[:, :],
                in_=bass.AP(tensor=bitsel, offset=0, ap=[[0, 128], [1, NI]]),
            )

            # ---------------- prep: load, norms, scale, transpose ----------------
            xj_r = xj[:, :].rearrange("(jb p) d -> p jb d", p=128)
            xi_r = xi[:, :].rearrange("(ib p) d -> p ib d", p=128)
            adjp_r = adjp[:, :].rearrange("(jb p) c -> p jb c", p=128)
            for g in range(NJB // 8):
                nc.scalar.dma_start(
                    out=xj_aug[g][:, :, 0:D], in_=xj_r[:, 8 * g:8 * (g + 1), :]
                )
                nc.vector.memset(xj_aug[g][:, :, D:D + 1], 1.0)
            nc.scalar.dma_start(out=adjp_sb[:, :, :], in_=adjp_r[:, :, :])

            xi_all = big.tile([128, NIB, D], BF16)
            nc.scalar.dma_start(out=xi_all[:, :, :], in_=xi_r[:, :, :])

            # row sums of squares via ACT Square + accum_out
            for jb in range(NJB):
                sq = prep.tile([128, D], F32, tag="sq")
                nc.scalar.activation(
                    out=sq[:, :], in_=xj_aug[jb // 8][:, jb % 8, 0:D],
                    func=mybir.ActivationFunctionType.Square,
                    accum_out=ssq_j[:, jb:jb + 1],
                )
            for ib in range(NIB):
                sq = prep.tile([128, D], F32, tag="sq")
                nc.scalar.activation(
                    out=sq[:, :], in_=xi_all[:, ib, :],
                    func=mybir.ActivationFunctionType.Square,
                    accum_out=ssq_i[:, ib:ib + 1],
                )
            # rj = 1/sqrt(ssq_j); ri = beta/sqrt(ssq_i)
            nc.scalar.activation(out=ssq_j[:, :], in_=ssq_j[:, :],
                                 func=mybir.ActivationFunctionType.Sqrt)
            nc.vector.reciprocal(out=rj[:, :], in_=ssq_j[:, :])
            nc.scalar.activation(out=ssq_i[:, :], in_=ssq_i[:, :],
                                 func=mybir.ActivationFunctionType.Sqrt)
            nc.vector.reciprocal(out=ri[:, :], in_=ssq_i[:, :])
            nc.vector.tensor_scalar(out=ri[:, :], in0=ri[:, :],
                                    scalar1=beta_sb[:, 0:1], scalar2=None,
                                    op0=mybir.AluOpType.mult)

            # scale rows then PE-transpose into xj_sT / xi_sT
            for jb in range(NJB):
                t = prep.tile([128, D], BF16, tag="xjs")
                nc.vector.tensor_scalar(out=t[:, :], in0=xj_aug[jb // 8][:, jb % 8, 0:D],
                                        scalar1=rj[:, jb:jb + 1], scalar2=None,
                                        op0=mybir.AluOpType.mult)
                for dh in range(DH):
                    tp = ps_t.tile([128, 128], BF16, tag="tp")
                    nc.tensor.matmul(tp[:, :], t[:, 128 * dh:128 * (dh + 1)],
                                     ident[:, :], is_transpose=True)
                    nc.vector.tensor_copy(
                        xj_sT[dh][:, 128 * jb:128 * (jb + 1)], tp[:, :])
            for ib in range(NIB):
                t = prep.tile([128, D], BF16, tag="xis")
                nc.vector.tensor_scalar(out=t[:, :], in0=xi_all[:, ib, :],
                                        scalar1=ri[:, ib:ib + 1], scalar2=None,
                                        op0=mybir.AluOpType.mult)
                for dh in range(DH):
                    tp = ps_t.tile([128, 128], BF16, tag="tp")
                    nc.tensor.matmul(tp[:, :], t[:, 128 * dh:128 * (dh + 1)],
                                     ident[:, :], is_transpose=True)
                    nc.vector.tensor_copy(
                        xi_sT[dh][:, 128 * ib:128 * (ib + 1)], tp[:, :])

            # ---------------- main loop ----------------
            for h in range(NHALF):
                ps_out = [ps_o.tile([128, D + 1], F32, name=f"ps_out_{h}_{bb}", tag="ps_out")
                          for bb in range(IBH)]
                for jb in range(NJB):
                    # MM1: ST[j=128, i=IW]
                    st = ps_s.tile([128, IW], F32, tag="st")
                    for dh in range(DH):
                        nc.tensor.matmul(
                            st[:, :],
                            xj_sT[dh][:, 128 * jb:128 * (jb + 1)],
                            xi_sT[dh][:, IW * h:IW * (h + 1)],
                            start=(dh == 0), stop=(dh == DH - 1),
                        )
                    e = work.tile([128, IW], F32, tag="e")
                    nc.scalar.activation(
                        out=e[:, :], in_=st[:, :],
                        func=mybir.ActivationFunctionType.Exp)
                    # mask bits: m[j, 8c+k] = adjT_byte[j, c] & (1 << k)
                    m = work.tile([128, IW], U8, tag="m")
                    byte_ap = adjp_sb[:, jb, h * IWB:(h + 1) * IWB]
                    rep8 = byte_ap.unsqueeze(2).broadcast_to([128, IWB, 8])
                    nc.vector.tensor_tensor(
                        out=m[:, :].rearrange("p (c k) -> p c k", k=8),
                        in0=rep8,
                        in1=bits_sb[:, IW * h:IW * (h + 1)].rearrange(
                            "p (c k) -> p c k", k=8),
                        op=mybir.AluOpType.bitwise_and,
                    )
                    # pu = e * m  (m in {0, 2^k}; the 2^k column factor cancels
                    # against the denominator in the final normalization)
                    pu = work.tile([128, IW], BF16, tag="pu")
                    nc.vector.tensor_tensor(
                        out=pu[:, :], in0=e[:, :], in1=m[:, :],
                        op=mybir.AluOpType.mult,
                    )
                    # MM2: out[i, :] += pu_tile.T @ xj_aug
                    for b in range(IBH):
                        nc.tensor.matmul(
                            ps_out[b][:, :],
                            pu[:, 128 * b:128 * (b + 1)],
                            xj_aug[jb // 8][:, jb % 8, :],
                            start=(jb == 0), stop=(jb == NJB - 1),
                        )
                # normalize + store
                for b in range(IBH):
                    ib = h * IBH + b
                    rden = outp.tile([128, 1], F32, tag="rden")
                    nc.vector.reciprocal(out=rden[:, :], in_=ps_out[b][:, D:D + 1])
                    of = outp.tile([128, D], BF16, tag="of")
                    nc.vector.tensor_scalar(
                        out=of[:, :], in0=ps_out[b][:, 0:D],
                        scalar1=rden[:, 0:1], scalar2=None,
                        op0=mybir.AluOpType.mult)
                    nc.scalar.dma_start(
                        out=out[128 * ib:128 * (ib + 1), :], in_=of[:, :])

    nc.finalize()
    return nc


_NC_CACHE = {}


def _get_nc(NI, M, D):
    key = (NI, M, D)
    if key not in _NC_CACHE:
        _NC_CACHE[key] = build_nc(NI, M, D)
    return _NC_CACHE[key]


def _prep_inputs(xi, xj, adj, beta):
    """Host-side layout/dtype prep (not device math): bf16 casts + bit-pack."""
    xi16 = np.asarray(xi, dtype=np.float32).astype(ml_dtypes.bfloat16)
    xj16 = np.asarray(xj, dtype=np.float32).astype(ml_dtypes.bfloat16)
    adjT = np.ascontiguousarray(np.asarray(adj, dtype=np.int32).T.astype(np.uint8))
    adjp = np.packbits(adjT, axis=1, bitorder="little")  # [M, N/8]
    beta = np.ascontiguousarray(np.asarray(beta, dtype=np.float32))
    return xi16, xj16, adjp, beta


def _make_in_maps(xi, xj, adj, beta):
    N, D = xi.shape
    NI = N // N_CORES
    xi16, xj16, adjp, beta = _prep_inputs(xi, xj, adj, beta)
    bitsel = np.tile(np.left_shift(np.ones(8, np.uint8), np.arange(8, dtype=np.uint8)),
                     NI // 8)
    return [
        {
            "xi": xi16[k * NI:(k + 1) * NI],
            "xj": xj16,
            "adjp": np.ascontiguousarray(adjp[:, k * (NI // 8):(k + 1) * (NI // 8)]),
            "bitsel": bitsel,
            "beta": beta,
        }
        for k in range(N_CORES)
    ]


def kernel(xi, xj, adj, beta):
    N, D = xi.shape
    M = xj.shape[0]
    NI = N // N_CORES
    nc = _get_nc(NI, M, D)
    in_maps = _make_in_maps(xi, xj, adj, beta)
    res = run_bass_kernel_spmd(nc, in_maps, list(range(N_CORES)))
    out16 = np.concatenate([res.results[k]["out"] for k in range(N_CORES)], axis=0)
    return out16.astype(np.float32)
